# revision 26
# baseline (speedup 1.0000x reference)
"""Distributed multi-head attention kernel for one TRN2 chip (8 NeuronCores).

Problem: nn_Attention_13048110645268
  x [2, 2048, 1024] f32 ->  attention(16 heads, d=64) -> out [2, 2048, 1024] f32

Sharding (Megatron-style batch x head-group):
  core c in [0,8): batch b = c//4, head group g = c%4 (heads 4g..4g+3).
  Each core computes qkv projections for its 4 heads, attention for those
  heads, then all-gathers the (unprojected) attention outputs within its
  4-core batch group and computes a 256-column slice of the output
  projection.  Host reassembles the full output (pure layout ops).

Per-core device pipeline (all matmuls bf16, accumulation fp32):
  qkT  [512,2048]  = (Wqk)^T x^T + bias      (feature-major)
  v    [2048,256]  = x Wv                    (token-major, lhsT = x^T tile)
  per query block qb (512 queries) / key tile kt (128 keys):
      S^T[kt,qt]   = k q^T  (per head, 2 heads row-packed, K=64)
      E = exp(S*scale) on ScalarE (PSUM->SBUF bf16), 2 instrs of [128,1024]
      e_acc       += E on DVE (bf16 2x mode); rowsum matmul once per
                     4-tile group instead of per key tile (4x fewer
                     PE streams); kt15 summed directly at the boundary
      out'^T[d,qt] += lhsT=v[kt,64], rhs=E^T  (2 heads col-packed)
      (av of key tile kt runs `lag` iterations behind so its operands are
       always ready; av is emitted BEFORE the next scores pair so the PE
       has ready work while ScalarE computes exp)
  normalize: out^T = out'^T * (1/rowsum); reciprocal_approx_fast on DVE
  AllGather outT per head-pair j: [128,512] -> [512,512], 2 collectives
  per query block (smaller payloads stay in the low-latency mesh regime)
  yT[256,2048] = Wp^T outT_full + beff  (fp32 output)

Host pre-restripes all weight/activation inputs so every big DMA is a
plain [128, N]-contiguous transfer (cheap descriptor generation).
"""

import os
import sys

import numpy as np

sys.path.insert(0, "/opt/trn_rl_repo")

import ml_dtypes  # noqa: E402

import concourse.bass as bass  # noqa: E402
import concourse.mybir as mybir  # noqa: E402
import concourse.tile as tile  # noqa: E402
from concourse import bacc  # noqa: E402
from concourse.bass_utils import run_bass_kernel_spmd  # noqa: E402

BF16 = mybir.dt.bfloat16
F32 = mybir.dt.float32
NBF16 = ml_dtypes.bfloat16

B, S, D = 2, 2048, 1024
H, HD = 16, 64
NCORES = 8
GROUPS = [[0, 1, 2, 3], [4, 5, 6, 7]]
HL = 4          # heads per core
DL = HL * HD    # 256 feature dims per core
P = 128
KT = S // P     # 16 key tiles
QB = 4          # query blocks
QW = S // QB    # 512 queries per block
KD = D // P     # 8 contraction tiles over model dim
SCALE = HD ** -0.5

_CACHE = {}


def _restripe(w):
    """[KD*128, C] -> [128, KD*C] with row p holding all kd-subtiles."""
    kd = w.shape[0] // P
    return np.ascontiguousarray(
        w.reshape(kd, P, w.shape[1]).transpose(1, 0, 2).reshape(P, -1))


def _emit(nc: bass.Bass, tc: tile.TileContext, xT, wqk, wv, wp, bqk, beff, yT):
    exp_fn = mybir.ActivationFunctionType.Exp

    with (
        tc.tile_pool(name="main", bufs=1) as mp,
        tc.tile_pool(name="ep", bufs=4) as ep,
        tc.tile_pool(name="eap", bufs=2) as eap,
        tc.tile_pool(name="gp", bufs=2) as gp,
        tc.tile_pool(name="yp", bufs=2) as yp,
        tc.tile_pool(name="rp", bufs=2) as rp,
        tc.tile_pool(name="ps_s", bufs=1, space="PSUM") as ps_s,
        tc.tile_pool(name="ps_acc", bufs=3, space="PSUM") as ps_acc,
        tc.tile_pool(name="ps_mm", bufs=1, space="PSUM") as ps_mm,
        tc.tile_pool(name="dram", bufs=2, space="DRAM") as dp,
    ):
        # ---------------- input DMA (ordered by first use) ----------------
        # wqk is packed ct-major on the host so the k/q column blocks the
        # first score pair needs can land before the rest of the weights
        # first-use tensors stream in kd-chunks so the first qk projection's
        # accumulation can start while the rest of the data is in flight
        wqk_sb = mp.tile([P, 4, KD, P], BF16)

        def _wqk_dma(ct, k0, k1):
            nc.sync.dma_start(
                wqk_sb[:, ct, k0:k1],
                wqk[:, ct * KD * P + k0 * P:ct * KD * P + k1 * P]
                .rearrange("p (kd c) -> p kd c", kd=k1 - k0))

        xT_sb = mp.tile([P, QB, KD, 512], BF16)   # x^T [d-part, n, d-tile, tok]

        def _xt_dma(n, k0, k1):
            nc.sync.dma_start(
                xT_sb[:, n, k0:k1],
                xT[n, :, k0 * 512:k1 * 512]
                .rearrange("p (kd u) -> p kd u", kd=k1 - k0))

        _wqk_dma(2, 0, 4)
        _xt_dma(0, 0, 2)
        _xt_dma(0, 2, 4)
        _wqk_dma(2, 4, 8)
        bqk_sb = mp.tile([P, 4], F32)
        nc.sync.dma_start(bqk_sb[:], bqk[:, :])
        _wqk_dma(0, 0, 4)
        _xt_dma(0, 4, 6)
        _xt_dma(0, 6, 8)
        _wqk_dma(0, 4, 8)
        wv_sb = mp.tile([P, KD, DL], BF16)
        nc.sync.dma_start(wv_sb[:],
                          wv[:, :].rearrange("p (kd c) -> p kd c", kd=KD))
        _wqk_dma(3, 0, 8)
        _wqk_dma(1, 0, 8)
        for n in range(1, QB):
            _xt_dma(n, 0, 8)
        wp_sb = mp.tile([P, KD, DL], BF16)
        nc.sync.dma_start(wp_sb[:],
                          wp[:, :].rearrange("p (kd c) -> p kd c", kd=KD))
        beff_sb = mp.tile([P, 2], F32)
        nc.sync.dma_start(beff_sb[:], beff[:, :])
        ones_sb = mp.tile([P, 64], BF16)
        nc.vector.memset(ones_sb[:], 1.0)
        warm_cc_in = dp.tile([8, 64], BF16, name="warm_cc_in")
        nc.sync.dma_start(warm_cc_in[:, :], xT[0, 0:8, 0:64])
        warm_cc_out = dp.tile([32, 64], BF16, name="warm_cc_out")
        nc.gpsimd.collective_compute(
            "AllGather",
            mybir.AluOpType.bypass,
            replica_groups=GROUPS,
            ins=[warm_cc_in[:, :].opt()],
            outs=[warm_cc_out[:, :].opt()],
        )

        # ---------------- qk projection: qkT_sb[c, t] ----------------
        # ct 0,1 = q (heads 0..3), ct 2,3 = k (heads 0..3).  Only the n=0
        # block is emitted up front; the rest is interleaved into attention
        # (deadline-scheduled) so ScalarE starts exp as early as possible.
        qkT_sb = mp.tile([P, 4, S], BF16)
        _qk_ps = {}

        def emit_qk(n, ct, pre=False, half=None):
            # pre-loop groups pipeline through the 3 'acc' slots (free until
            # the first av/rs allocation); in-loop groups must use the
            # rotating 'mm' slot to avoid deadlocking against the qb-long
            # accumulator tiles.  half=0/1 emits 4 of the 8 contraction
            # steps so in-loop groups don't lump 3.4us of PE work into one
            # iteration (the psum tile persists across the two halves).
            if half in (None, 0):
                if pre:
                    ps_qk = ps_acc.tile([P, 512], F32, tag="acc", name="ps_qk")
                else:
                    ps_qk = ps_mm.tile([P, 512], F32, tag="mm", name="ps_qk")
                _qk_ps[(n, ct)] = ps_qk
            else:
                ps_qk = _qk_ps[(n, ct)]
            kds = range(KD) if half is None else range(half * 4, half * 4 + 4)
            for kd in kds:
                nc.tensor.matmul(
                    ps_qk[:],
                    lhsT=wqk_sb[:, ct, kd, :],
                    rhs=xT_sb[:, n, kd, :],
                    start=(kd == 0),
                    stop=(kd == KD - 1),
                )
            if half in (None, 1):
                nc.vector.tensor_scalar_add(
                    qkT_sb[:, ct, n * 512:(n + 1) * 512], ps_qk[:],
                    bqk_sb[:, ct:ct + 1],
                )

        # PE warm-up: dummy matmuls with no input deps run while the input
        # DMAs land, lifting the HAM clock gate to 8/8 before real work
        warm_sb = mp.tile([P, 512], BF16)
        nc.vector.memset(warm_sb[:], 1.0)
        ps_warm = ps_s.tile([P, 2 * 512], F32, name="ps_warm", tag="sc0")
        for w in range(10):
            nc.tensor.matmul(
                ps_warm[:, (w % 2) * 512:(w % 2 + 1) * 512],
                lhsT=warm_sb[:, 0:P],
                rhs=warm_sb[:, :],
                start=True,
                stop=True,
            )

        for ct in (2, 0):   # k,q of head-pair 0 first: earliest expA
            emit_qk(0, ct, pre=True)

        # ---------------- attention + AG + proj, per query block ----------------
        v_sb = mp.tile([P, KT, DL], BF16)
        outT_sb = mp.tile([P, QB, 2, 512], BF16)  # pair j: heads 2j (p<64), 2j+1
        g_tiles = [None] * QB

        _vpair = [None]

        def emit_v(tt):
            if tt % 2 == 0:
                _vpair[0] = ps_mm.tile([P, 512], F32, tag="mm", name="ps_v")
            half = (tt % 2) * DL
            ps_v = _vpair[0]
            for kd in range(KD):
                nc.tensor.matmul(
                    ps_v[:, half:half + DL],
                    lhsT=xT_sb[:, tt // 4, kd, (tt % 4) * P:(tt % 4 + 1) * P],
                    rhs=wv_sb[:, kd, :],
                    start=(kd == 0),
                    stop=(kd == KD - 1),
                )
            nc.vector.tensor_copy(v_sb[:, tt, :], ps_v[:, half:half + DL])

        _pj_ps = {}

        def emit_proj_half(qb, j, half=None):
            # the gathered activations land per rank-block r (kd = 2r+j'),
        # so the natural kd order consumes them as the g DMAs complete;
            # half=0/1 spreads the 8 steps over two filler slots
            qs = qb * QW
            if half in (None, 0):
                ps_y = ps_mm.tile([P, 512], F32, tag="mm", name="ps_y")
                _pj_ps[(qb, j)] = ps_y
            else:
                ps_y = _pj_ps[(qb, j)]
            kds = range(KD) if half is None else range(half * 4, half * 4 + 4)
            for kd in kds:
                nc.tensor.matmul(
                    ps_y[:],
                    lhsT=wp_sb[:, kd, j * P:(j + 1) * P],
                    rhs=g_tiles[qb][:, kd // 2, kd % 2, :],
                    start=(kd == 0),
                    stop=(kd == KD - 1),
                )
            if half in (None, 1):
                y_sb = yp.tile([P, 512], F32, name="y_sb")
                nc.vector.tensor_scalar_add(y_sb[:], ps_y[:],
                                            beff_sb[:, j:j + 1])
                nc.sync.dma_start(yT[j * P:(j + 1) * P, qs:qs + QW], y_sb[:])

        def emit_av_pair(kt, e_sb, ps_av, pair):
            for hh in range(2):
                h = 2 * pair + hh
                nc.tensor.matmul(
                    ps_av[64 * hh:64 * hh + HD, :],
                    lhsT=v_sb[:, kt, h * HD:(h + 1) * HD],
                    rhs=e_sb[:, h * 512:(h + 1) * 512],
                    start=(kt == 0),
                    stop=(kt == KT - 1),
                )

        def emit_rs_acc(e_src, ps_rs, start, stop):
            for h in range(HL):
                nc.tensor.matmul(
                    ps_rs[32 * h:32 * h + 1, :],
                    lhsT=ones_sb[:, 0:1],
                    rhs=e_src[:, h * 512:(h + 1) * 512],
                    start=start,
                    stop=stop,
                    tile_position=(0, 32 * h),
                )

        def make_norm_pair(qb, j, o_sb, r_sb):
            def _norm():
                rb_ps = ps_mm.tile([P, 512], F32, tag="mm", name="rb_ps")
                for hh in range(2):
                    h = 2 * j + hh
                    nc.tensor.matmul(
                        rb_ps[64 * hh:64 * hh + 64, :],
                        lhsT=ones_sb[32 * h:32 * h + 1, :],
                        rhs=r_sb[32 * h:32 * h + 1, :],
                        start=True,
                        stop=True,
                        tile_position=(32 * h, 64 * hh),
                    )
                nc.vector.tensor_mul(outT_sb[:, qb, j, :], o_sb[:],
                                     rb_ps[:])
            return _norm

        def make_ag(qb):
            # one AllGather per query block ([256,512] -> [1024,512]): a
            # single rendezvous on the serial CC queue.  The DRAM->SBUF
            # unpack is split per rank-block so the projection's kd
            # accumulation streams in as each 256 KB block lands.
            def _ag():
                cc_in = dp.tile([2 * P, QW], BF16, name="cc_in")
                nc.sync.dma_start(cc_in[:, :].rearrange("(j p) t -> p j t", p=P),
                                  outT_sb[:, qb])
                cc_out = dp.tile([D, QW], BF16, name="cc_out")
                nc.gpsimd.collective_compute(
                    "AllGather",
                    mybir.AluOpType.bypass,
                    replica_groups=GROUPS,
                    ins=[cc_in[:, :].opt()],
                    outs=[cc_out[:, :].opt()],
                )
                g_sb = gp.tile([P, KD // 2, 2, QW], BF16, name="g_sb")
                g_tiles[qb] = g_sb
                for r in range(4):
                    nc.sync.dma_start(
                        g_sb[:, r],
                        cc_out[r * 2 * P:(r + 1) * 2 * P, :]
                        .rearrange("(j p) t -> p j t", p=P))
            return _ag

        # Deadline-scheduled PE filler for each (qb, kt) iteration:
        #  - qb0 carries the remaining qk blocks (k tiles via the acc pool
        #    before the lag-3 accumulators are allocated) and all v tiles
        #  - qb>=1 carry the q blocks for later qbs, the normalization +
        #    AllGather of qb-1 (kt1/kt2), and proj of qb-1 (kt8/kt10)
        filler = {qb: {} for qb in range(QB)}

        def _add(qb, kt, fn):
            filler[qb].setdefault(kt, []).append(fn)

        _add(0, 0, lambda: emit_qk(1, 2, pre=True))
        _add(0, 1, lambda: emit_qk(1, 3, pre=True))
        _add(0, 1, lambda: emit_qk(2, 2, pre=True))
        _add(0, 2, lambda: emit_qk(2, 3, pre=True))
        _add(0, 2, lambda: emit_qk(3, 2, pre=True))
        _add(0, 2, lambda: emit_qk(3, 3, pre=True))
        _v_sched = {_t: [_t] for _t in range(12)}
        _v_sched[11] = [11, 12]
        _v_sched[12] = [13, 14]
        _v_sched[13] = [15]
        for _kt, _ts in _v_sched.items():
            for _t in _ts:
                _add(0, _kt, lambda t=_t: emit_v(t))
        _add(0, 9, lambda: emit_qk(1, 0, half=0))
        _add(0, 10, lambda: emit_qk(1, 0, half=1))
        _add(0, 12, lambda: emit_qk(1, 1, half=0))
        _add(0, 13, lambda: emit_qk(1, 1, half=1))
        for _q, _n in ((1, 2), (2, 3)):
            _add(_q, 3, lambda n=_n: emit_qk(n, 0, half=0))
            _add(_q, 4, lambda n=_n: emit_qk(n, 0, half=1))
            _add(_q, 5, lambda n=_n: emit_qk(n, 1, half=0))
            _add(_q, 6, lambda n=_n: emit_qk(n, 1, half=1))
        for _qb in (2, 3):
            _add(_qb, 8, lambda q=_qb: emit_proj_half(q - 2, 0, half=0))
            _add(_qb, 9, lambda q=_qb: emit_proj_half(q - 2, 0, half=1))
            _add(_qb, 11, lambda q=_qb: emit_proj_half(q - 2, 1, half=0))
            _add(_qb, 12, lambda q=_qb: emit_proj_half(q - 2, 1, half=1))

        def emit_scores_pair(ps_sc, qb, kt, pair):
            qs = qb * QW
            for hh in range(2):
                h = 2 * pair + hh
                hp = (HD * h) % P                 # 0, 64, 0, 64
                hc = h // 2                       # q ctile; k ctile = 2 + hc
                nc.tensor.matmul(
                    ps_sc[:, hh * 512:(hh + 1) * 512],
                    lhsT=qkT_sb[hp:hp + HD, 2 + hc, kt * P:(kt + 1) * P],
                    rhs=qkT_sb[hp:hp + HD, hc, qs:qs + QW],
                    start=True,
                    stop=True,
                )

        def new_sc():
            # one 2-bank PSUM tile per head pair: exp(pair, kt) and
            # scores(pair, kt+1) of DIFFERENT pairs overlap, breaking the
            # full-tile exp<->scores serialization of a single 4-bank tile
            return (ps_s.tile([P, 1024], F32, name="ps_sc0", tag="sc0"),
                    ps_s.tile([P, 1024], F32, name="ps_sc1", tag="sc1"))

        # first scores pair right after its two qk groups; pair 1's qk
        # groups land while ScalarE already runs expA(kt0)
        sc_next = new_sc()
        emit_scores_pair(sc_next[0], 0, 0, 0)
        emit_qk(0, 3, pre=True)
        emit_qk(0, 1, pre=True)
        emit_scores_pair(sc_next[1], 0, 0, 1)
        for qb in range(QB):
            qs = qb * QW
            lag = 1 if qb == QB - 1 else 3
            ps_av0 = ps_av1 = ps_rs = None
            pending = []
            eacc = {}

            if sc_next is None:
                sc_next = new_sc()
                emit_scores_pair(sc_next[0], qb, 0, 0)
                emit_scores_pair(sc_next[1], qb, 0, 1)
            sc_cur = sc_next
            sc_next = None

            for kt in range(KT):
                e_sb = ep.tile([P, 4 * 512], BF16, name="e_sb")
                nc.scalar.activation(e_sb[:, 0:1024], sc_cur[0][:], exp_fn,
                                     scale=SCALE)
                nc.scalar.activation(e_sb[:, 1024:2048], sc_cur[1][:],
                                     exp_fn, scale=SCALE)
                # group-accumulate exp tiles on DVE (groups kt 0-7 and
                # 8-14; kt15 is summed directly at the boundary)
                g = kt // 8
                if kt % 8 == 0 and kt < 15:
                    eacc[g] = eap.tile([P, 4 * 512], BF16, name="eacc")
                    nc.vector.tensor_copy(eacc[g][:], e_sb[:])
                elif kt < 15:
                    nc.vector.tensor_add(eacc[g][:], eacc[g][:], e_sb[:])
                nxt = kt + 1 < KT
                if nxt:
                    sc_nx = new_sc()
                # lagged av first: its operands are ready, so the PE works
                # through it while ScalarE is still on this tile's exp
                pending.append((kt, e_sb))
                if len(pending) > lag:
                    if ps_av0 is None:
                        ps_av0 = ps_acc.tile([P, 512], F32, tag="acc",
                                             name="ps_av0")
                        ps_av1 = ps_acc.tile([P, 512], F32, tag="acc",
                                             name="ps_av1")
                        ps_rs = ps_acc.tile([P, 512], F32, tag="acc",
                                            name="ps_rs")
                    k0, e0 = pending.pop(0)
                    emit_av_pair(k0, e0, ps_av0, 0)
                    emit_av_pair(k0, e0, ps_av1, 1)
                if kt in (8, 15):
                    gg = 0 if kt == 8 else 1
                    emit_rs_acc(eacc[gg], ps_rs, start=(gg == 0), stop=False)
                if nxt:
                    emit_scores_pair(sc_nx[0], qb, kt + 1, 0)
                    emit_scores_pair(sc_nx[1], qb, kt + 1, 1)
                for fn in filler[qb].get(kt, ()):
                    fn()
                if nxt:
                    sc_cur = sc_nx
            e15 = pending[-1][1]
            for k0, e0 in pending:
                emit_av_pair(k0, e0, ps_av0, 0)
                emit_av_pair(k0, e0, ps_av1, 1)
            emit_rs_acc(e15, ps_rs, start=False, stop=True)

            if qb + 1 < QB:
                sc_next = new_sc()
                emit_scores_pair(sc_next[0], qb + 1, 0, 0)
                emit_scores_pair(sc_next[1], qb + 1, 0, 1)

            # release the accumulator PSUM slots fast: raw copies to SBUF
            o_sb = [rp.tile([P, 512], BF16, name="o0_sb"),
                    rp.tile([P, 512], BF16, name="o1_sb")]
            if qb == QB - 1:   # tail: ScalarE is idle, DVE does recip in parallel
                nc.scalar.copy(o_sb[0][:], ps_av0[:])
                nc.scalar.copy(o_sb[1][:], ps_av1[:])
            else:
                nc.vector.tensor_copy(o_sb[0][:], ps_av0[:])
                nc.vector.tensor_copy(o_sb[1][:], ps_av1[:])
            rf_sb = rp.tile([P, 512], F32, name="rf_sb")
            nc.vector.reciprocal_approx_fast(out=rf_sb[:], in_=ps_rs[:])
            r_sb = rp.tile([P, 512], BF16, name="r_sb")
            nc.vector.tensor_copy(r_sb[:], rf_sb[:])

            if qb < QB - 1:
                _add(qb + 1, 1, make_norm_pair(qb, 0, o_sb[0], r_sb))
                _add(qb + 1, 2, make_norm_pair(qb, 1, o_sb[1], r_sb))
                _add(qb + 1, 2, make_ag(qb))
            else:
                make_norm_pair(qb, 0, o_sb[0], r_sb)()
                make_norm_pair(qb, 1, o_sb[1], r_sb)()
                make_ag(qb)()

        emit_proj_half(QB - 2, 0)
        emit_proj_half(QB - 2, 1)
        emit_proj_half(QB - 1, 0)
        emit_proj_half(QB - 1, 1)


def _build():
    if "nc" in _CACHE:
        return _CACHE["nc"]
    nc = bacc.Bacc(
        "TRN2",
        target_bir_lowering=False,
        debug=False,
        num_devices=NCORES,
    )
    xT = nc.declare_dram_parameter("xT", [QB, P, KD * 512], BF16, isOutput=False)
    wqk = nc.declare_dram_parameter("wqk", [P, KD * 2 * DL], BF16, isOutput=False)
    wv = nc.declare_dram_parameter("wv", [P, KD * DL], BF16, isOutput=False)
    wp = nc.declare_dram_parameter("wp", [P, KD * DL], BF16, isOutput=False)
    bqk = nc.declare_dram_parameter("bqk", [P, 4], F32, isOutput=False)
    beff = nc.declare_dram_parameter("beff", [P, 2], F32, isOutput=False)
    yT = nc.declare_dram_parameter("yT", [DL, S], F32, isOutput=True)

    with tile.TileContext(nc) as tc:
        _emit(nc, tc, xT, wqk, wv, wp, bqk, beff, yT)
    nc.compile()
    _CACHE["nc"] = nc
    return nc


def kernel(x, W_qkv, b_qkv, W_proj, b_proj):
    x = np.asarray(x, np.float32)
    W_qkv = np.asarray(W_qkv, np.float32)
    b_qkv = np.asarray(b_qkv, np.float32)
    W_proj = np.asarray(W_proj, np.float32)
    b_proj = np.asarray(b_proj, np.float32)

    nc = _build()

    b_v = b_qkv[2 * D:3 * D]
    xTt = {}
    for b in range(B):
        xT = np.ascontiguousarray(x[b].T)            # [1024, 2048]
        t = xT.reshape(KD, P, QB, 512).transpose(2, 1, 0, 3)
        xTt[b] = np.ascontiguousarray(t.reshape(QB, P, KD * 512)).astype(NBF16)

    in_maps = []
    for c in range(NCORES):
        b, g = divmod(c, 4)
        cs = DL * g
        wqk_c = np.concatenate(
            [W_qkv[:, cs:cs + DL], W_qkv[:, D + cs:D + cs + DL]], axis=1)
        # pack ct-major ([q0,q1,k0,k1] column blocks of 128) so the device
        # can DMA each block separately, earliest-needed first
        wqk_p = np.concatenate(
            [_restripe(wqk_c[:, ct * P:(ct + 1) * P]) for ct in range(4)],
            axis=1)
        bqk_c = np.concatenate(
            [b_qkv[cs:cs + DL], b_qkv[D + cs:D + cs + DL]]).reshape(4, P).T
        beff_c = (b_v @ W_proj[:, cs:cs + DL] + b_proj[cs:cs + DL]).reshape(2, P).T
        in_maps.append({
            "xT": xTt[b],
            "wqk": np.ascontiguousarray(wqk_p).astype(NBF16),
            "wv": _restripe(W_qkv[:, 2 * D + cs:2 * D + cs + DL]).astype(NBF16),
            "wp": _restripe(W_proj[:, cs:cs + DL]).astype(NBF16),
            "bqk": np.ascontiguousarray(bqk_c, np.float32),
            "beff": np.ascontiguousarray(beff_c, np.float32),
        })

    trace = bool(int(os.environ.get("TRN_KERNEL_TRACE", "0")))
    res = run_bass_kernel_spmd(nc, in_maps, core_ids=list(range(NCORES)),
                               trace=trace)
    if trace and res.exec_time_ns is not None:
        print(f"HW exec time: {res.exec_time_ns} ns", flush=True)
    _CACHE["last_result"] = res

    out = np.empty((B, S, D), np.float32)
    for c in range(NCORES):
        b, g = divmod(c, 4)
        out[b, :, DL * g:DL * (g + 1)] = res.results[c]["yT"].T
    return out


# revision 32
# speedup vs baseline: 1.0263x; 1.0263x over previous
"""Distributed multi-head attention kernel for one TRN2 chip (8 NeuronCores).

Problem: nn_Attention_13048110645268
  x [2, 2048, 1024] f32 ->  attention(16 heads, d=64) -> out [2, 2048, 1024] f32

Sharding (Megatron-style batch x head-group):
  core c in [0,8): batch b = c//4, head group g = c%4 (heads 4g..4g+3).
  Each core computes qkv projections for its 4 heads, attention for those
  heads, then all-gathers the (unprojected) attention outputs within its
  4-core batch group and computes a 256-column slice of the output
  projection.  Host reassembles the full output (pure layout ops).

Per-core device pipeline (all matmuls bf16, accumulation fp32):
  qkT  [512,2048]  = (Wqk)^T x^T + bias      (feature-major)
  v    [2048,256]  = x Wv                    (token-major, lhsT = x^T tile)
  per query block qb (512 queries) / key tile kt (128 keys):
      S^T[kt,qt]   = k q^T  (per head, 2 heads row-packed, K=64)
      E = exp(S*scale) on ScalarE (PSUM->SBUF bf16), 2 instrs of [128,1024]
      e_acc       += E on DVE (bf16 2x mode); rowsum matmul once per
                     4-tile group instead of per key tile (4x fewer
                     PE streams); kt15 summed directly at the boundary
      out'^T[d,qt] += lhsT=v[kt,64], rhs=E^T  (2 heads col-packed)
      (av of key tile kt runs `lag` iterations behind so its operands are
       always ready; av is emitted BEFORE the next scores pair so the PE
       has ready work while ScalarE computes exp)
  normalize: out^T = out'^T * (1/rowsum); reciprocal_approx_fast on DVE
  AllGather outT per head-pair j: [128,512] -> [512,512], 2 collectives
  per query block (smaller payloads stay in the low-latency mesh regime)
  yT[256,2048] = Wp^T outT_full + beff  (fp32 output)

Host pre-restripes all weight/activation inputs so every big DMA is a
plain [128, N]-contiguous transfer (cheap descriptor generation).
"""

import os
import sys

import numpy as np

sys.path.insert(0, "/opt/trn_rl_repo")

import ml_dtypes  # noqa: E402

import concourse.bass as bass  # noqa: E402
import concourse.mybir as mybir  # noqa: E402
import concourse.tile as tile  # noqa: E402
from concourse import bacc  # noqa: E402
from concourse.bass_utils import run_bass_kernel_spmd  # noqa: E402

BF16 = mybir.dt.bfloat16
F32 = mybir.dt.float32
NBF16 = ml_dtypes.bfloat16

B, S, D = 2, 2048, 1024
H, HD = 16, 64
NCORES = 8
GROUPS = [[0, 1, 2, 3], [4, 5, 6, 7]]
HL = 4          # heads per core
DL = HL * HD    # 256 feature dims per core
P = 128
KT = S // P     # 16 key tiles
QB = 4          # query blocks
QW = S // QB    # 512 queries per block
KD = D // P     # 8 contraction tiles over model dim
SCALE = HD ** -0.5

_CACHE = {}


def _restripe(w):
    """[KD*128, C] -> [128, KD*C] with row p holding all kd-subtiles."""
    kd = w.shape[0] // P
    return np.ascontiguousarray(
        w.reshape(kd, P, w.shape[1]).transpose(1, 0, 2).reshape(P, -1))


def _emit(nc: bass.Bass, tc: tile.TileContext, xT, wqk, wv, wp, bqk, beff, yT):
    exp_fn = mybir.ActivationFunctionType.Exp

    with (
        tc.tile_pool(name="main", bufs=1) as mp,
        tc.tile_pool(name="ep", bufs=4) as ep,
        tc.tile_pool(name="eap", bufs=2) as eap,
        tc.tile_pool(name="gp", bufs=2) as gp,
        tc.tile_pool(name="yp", bufs=2) as yp,
        tc.tile_pool(name="rp", bufs=2) as rp,
        tc.tile_pool(name="ps_s", bufs=1, space="PSUM") as ps_s,
        tc.tile_pool(name="ps_acc", bufs=3, space="PSUM") as ps_acc,
        tc.tile_pool(name="ps_mm", bufs=1, space="PSUM") as ps_mm,
        tc.tile_pool(name="dram", bufs=2, space="DRAM") as dp,
    ):
        # ---------------- input DMA (ordered by first use) ----------------
        # wqk is packed ct-major on the host so the k/q column blocks the
        # first score pair needs can land before the rest of the weights
        # first-use tensors stream in kd-chunks so the first qk projection's
        # accumulation can start while the rest of the data is in flight
        wqk_sb = mp.tile([P, 4, KD, P], BF16)

        def _wqk_dma(ct, k0, k1):
            nc.sync.dma_start(
                wqk_sb[:, ct, k0:k1],
                wqk[:, ct * KD * P + k0 * P:ct * KD * P + k1 * P]
                .rearrange("p (kd c) -> p kd c", kd=k1 - k0))

        xT_sb = mp.tile([P, QB, KD, 512], BF16)   # x^T [d-part, n, d-tile, tok]

        def _xt_dma(n, k0, k1):
            nc.sync.dma_start(
                xT_sb[:, n, k0:k1],
                xT[n, :, k0 * 512:k1 * 512]
                .rearrange("p (kd u) -> p kd u", kd=k1 - k0))

        _wqk_dma(2, 0, 4)
        _xt_dma(0, 0, 2)
        _xt_dma(0, 2, 4)
        _wqk_dma(2, 4, 8)
        bqk_sb = mp.tile([P, 4], F32)
        nc.sync.dma_start(bqk_sb[:], bqk[:, :])
        _wqk_dma(0, 0, 4)
        _xt_dma(0, 4, 6)
        _xt_dma(0, 6, 8)
        _wqk_dma(0, 4, 8)
        wv_sb = mp.tile([P, KD, DL], BF16)
        nc.sync.dma_start(wv_sb[:],
                          wv[:, :].rearrange("p (kd c) -> p kd c", kd=KD))
        _wqk_dma(3, 0, 8)
        _wqk_dma(1, 0, 8)
        for n in range(1, QB):
            _xt_dma(n, 0, 8)
        wp_sb = mp.tile([P, KD, DL], BF16)
        nc.sync.dma_start(wp_sb[:],
                          wp[:, :].rearrange("p (kd c) -> p kd c", kd=KD))
        beff_sb = mp.tile([P, 2], F32)
        nc.sync.dma_start(beff_sb[:], beff[:, :])
        ones_sb = mp.tile([P, 64], BF16)
        nc.vector.memset(ones_sb[:], 1.0)
        warm_cc_in = dp.tile([8, 64], BF16, name="warm_cc_in")
        nc.sync.dma_start(warm_cc_in[:, :], xT[0, 0:8, 0:64])
        warm_cc_out = dp.tile([32, 64], BF16, name="warm_cc_out")
        nc.gpsimd.collective_compute(
            "AllGather",
            mybir.AluOpType.bypass,
            replica_groups=GROUPS,
            ins=[warm_cc_in[:, :].opt()],
            outs=[warm_cc_out[:, :].opt()],
        )

        # ---------------- qk projection: qkT_sb[c, t] ----------------
        # ct 0,1 = q (heads 0..3), ct 2,3 = k (heads 0..3).  Only the n=0
        # block is emitted up front; the rest is interleaved into attention
        # (deadline-scheduled) so ScalarE starts exp as early as possible.
        qkT_sb = mp.tile([P, 4, S], BF16)
        _qk_ps = {}

        def emit_qk(n, ct, pre=False, half=None):
            # pre-loop groups pipeline through the 3 'acc' slots (free until
            # the first av/rs allocation); in-loop groups must use the
            # rotating 'mm' slot to avoid deadlocking against the qb-long
            # accumulator tiles.  half=0/1 emits 4 of the 8 contraction
            # steps so in-loop groups don't lump 3.4us of PE work into one
            # iteration (the psum tile persists across the two halves).
            if half in (None, 0):
                if pre:
                    ps_qk = ps_acc.tile([P, 512], F32, tag="acc", name="ps_qk")
                else:
                    ps_qk = ps_mm.tile([P, 512], F32, tag="mm", name="ps_qk")
                _qk_ps[(n, ct)] = ps_qk
            else:
                ps_qk = _qk_ps[(n, ct)]
            kds = range(KD) if half is None else range(half * 4, half * 4 + 4)
            for kd in kds:
                nc.tensor.matmul(
                    ps_qk[:],
                    lhsT=wqk_sb[:, ct, kd, :],
                    rhs=xT_sb[:, n, kd, :],
                    start=(kd == 0),
                    stop=(kd == KD - 1),
                )
            if half in (None, 1):
                nc.vector.tensor_scalar_add(
                    qkT_sb[:, ct, n * 512:(n + 1) * 512], ps_qk[:],
                    bqk_sb[:, ct:ct + 1],
                )

        # PE warm-up: dummy matmuls with no input deps run while the input
        # DMAs land, lifting the HAM clock gate to 8/8 before real work
        warm_sb = mp.tile([P, 512], BF16)
        nc.vector.memset(warm_sb[:], 1.0)
        ps_warm = ps_s.tile([P, 2 * 512], F32, name="ps_warm", tag="sc0")
        for w in range(10):
            nc.tensor.matmul(
                ps_warm[:, (w % 2) * 512:(w % 2 + 1) * 512],
                lhsT=warm_sb[:, 0:P],
                rhs=warm_sb[:, :],
                start=True,
                stop=True,
            )

        for ct in (2, 0):   # k,q of head-pair 0 first: earliest expA
            emit_qk(0, ct, pre=True)

        # ---------------- attention + AG + proj, per query block ----------------
        v_sb = mp.tile([P, KT, DL], BF16)
        outT_sb = mp.tile([P, QB, 2, 512], BF16)  # pair j: heads 2j (p<64), 2j+1
        g_tiles = [[None, None] for _ in range(QB)]

        _vpair = [None]

        def emit_v(tt):
            if tt % 2 == 0:
                _vpair[0] = ps_mm.tile([P, 512], F32, tag="mm", name="ps_v")
            half = (tt % 2) * DL
            ps_v = _vpair[0]
            for kd in range(KD):
                nc.tensor.matmul(
                    ps_v[:, half:half + DL],
                    lhsT=xT_sb[:, tt // 4, kd, (tt % 4) * P:(tt % 4 + 1) * P],
                    rhs=wv_sb[:, kd, :],
                    start=(kd == 0),
                    stop=(kd == KD - 1),
                )
            nc.vector.tensor_copy(v_sb[:, tt, :], ps_v[:, half:half + DL])

        _pj_ps = {}

        def emit_proj_half(qb, j, half=None, tail=False):
            # half=0/1 spreads the 8 steps over two filler slots; the tail
            # pair uses the (now free) accumulator slots so both output
            # halves can make progress concurrently
            qs = qb * QW
            if half in (None, 0):
                if tail:
                    ps_y = ps_acc.tile([P, 512], F32, tag="acc", name="ps_y")
                else:
                    ps_y = ps_mm.tile([P, 512], F32, tag="mm", name="ps_y")
                _pj_ps[(qb, j)] = ps_y
            else:
                ps_y = _pj_ps[(qb, j)]
            # half 0 = even kd (j=0 gather), half 1 = odd kd (j=1 gather):
            # the tail projections start as soon as the first AG lands
            if half is None:
                kds = [0, 2, 4, 6, 1, 3, 5, 7]
            else:
                kds = [2 * r + half for r in range(4)]
            for idx, kd in enumerate(kds):
                first = kd == 0
                last = kd == 7
                nc.tensor.matmul(
                    ps_y[:],
                    lhsT=wp_sb[:, kd, j * P:(j + 1) * P],
                    rhs=g_tiles[qb][kd % 2][:, kd // 2, :],
                    start=first,
                    stop=last,
                )
            if half in (None, 1):
                y_sb = yp.tile([P, 512], F32, name="y_sb")
                nc.vector.tensor_scalar_add(y_sb[:], ps_y[:],
                                            beff_sb[:, j:j + 1])
                nc.sync.dma_start(yT[j * P:(j + 1) * P, qs:qs + QW], y_sb[:])

        def emit_av_pair(kt, e_sb, ps_av, pair):
            for hh in range(2):
                h = 2 * pair + hh
                nc.tensor.matmul(
                    ps_av[64 * hh:64 * hh + HD, :],
                    lhsT=v_sb[:, kt, h * HD:(h + 1) * HD],
                    rhs=e_sb[:, h * 512:(h + 1) * 512],
                    start=(kt == 0),
                    stop=(kt == KT - 1),
                )

        def emit_rs_acc(e_src, ps_rs, start, stop):
            for h in range(HL):
                nc.tensor.matmul(
                    ps_rs[32 * h:32 * h + 1, :],
                    lhsT=ones_sb[:, 0:1],
                    rhs=e_src[:, h * 512:(h + 1) * 512],
                    start=start,
                    stop=stop,
                    tile_position=(0, 32 * h),
                )

        def make_norm_pair(qb, j, o_sb, r_sb):
            def _norm():
                rb_ps = ps_mm.tile([P, 512], F32, tag="mm", name="rb_ps")
                for hh in range(2):
                    h = 2 * j + hh
                    nc.tensor.matmul(
                        rb_ps[64 * hh:64 * hh + 64, :],
                        lhsT=ones_sb[32 * h:32 * h + 1, :],
                        rhs=r_sb[32 * h:32 * h + 1, :],
                        start=True,
                        stop=True,
                        tile_position=(32 * h, 64 * hh),
                    )
                nc.vector.tensor_mul(outT_sb[:, qb, j, :], o_sb[:],
                                     rb_ps[:])
            return _norm

        def make_ag_half(qb, j):
            # AllGather one head-pair [128,512] -> [512,512]: 512 KB stays
            # on the fast small-message path (a fused 1 MB gather costs
            # ~24us on the ring vs ~2x10us split).  The DRAM->SBUF unpack
            # is per rank-block (plain [128,512] copies) so the projection
            # can consume each 128 KB block as it lands.
            def _ag():
                cc_in = dp.tile([P, QW], BF16, name=f"cc_in{j}")
                nc.sync.dma_start(cc_in[:, :], outT_sb[:, qb, j])
                cc_out = dp.tile([4 * P, QW], BF16, name=f"cc_out{j}")
                nc.gpsimd.collective_compute(
                    "AllGather",
                    mybir.AluOpType.bypass,
                    replica_groups=GROUPS,
                    ins=[cc_in[:, :].opt()],
                    outs=[cc_out[:, :].opt()],
                )
                g_sb = gp.tile([P, KD // 2, QW], BF16, name=f"g_sb{j}")
                g_tiles[qb][j] = g_sb
                for r in range(4):
                    nc.sync.dma_start(g_sb[:, r, :],
                                      cc_out[r * P:(r + 1) * P, :])
            return _ag

        # Deadline-scheduled PE filler for each (qb, kt) iteration:
        #  - qb0 carries the remaining qk blocks (k tiles via the acc pool
        #    before the lag-3 accumulators are allocated) and all v tiles
        #  - qb>=1 carry the q blocks for later qbs, the normalization +
        #    AllGather of qb-1 (kt1/kt2), and proj of qb-1 (kt8/kt10)
        filler = {qb: {} for qb in range(QB)}

        def _add(qb, kt, fn):
            filler[qb].setdefault(kt, []).append(fn)

        _add(0, 0, lambda: emit_qk(1, 2, pre=True))
        _add(0, 1, lambda: emit_qk(1, 3, pre=True))
        _add(0, 1, lambda: emit_qk(2, 2, pre=True))
        _add(0, 2, lambda: emit_qk(2, 3, pre=True))
        _add(0, 2, lambda: emit_qk(3, 2, pre=True))
        _add(0, 2, lambda: emit_qk(3, 3, pre=True))
        _v_sched = {_t: [_t] for _t in range(12)}
        _v_sched[11] = [11, 12]
        _v_sched[12] = [13, 14]
        _v_sched[13] = [15]
        for _kt, _ts in _v_sched.items():
            for _t in _ts:
                _add(0, _kt, lambda t=_t: emit_v(t))
        _add(0, 9, lambda: emit_qk(1, 0, half=0))
        _add(0, 10, lambda: emit_qk(1, 0, half=1))
        _add(0, 12, lambda: emit_qk(1, 1, half=0))
        _add(0, 13, lambda: emit_qk(1, 1, half=1))
        for _q, _n in ((1, 2), (2, 3)):
            _add(_q, 3, lambda n=_n: emit_qk(n, 0, half=0))
            _add(_q, 4, lambda n=_n: emit_qk(n, 0, half=1))
            _add(_q, 5, lambda n=_n: emit_qk(n, 1, half=0))
            _add(_q, 6, lambda n=_n: emit_qk(n, 1, half=1))
        for _qb in (2, 3):
            _add(_qb, 8, lambda q=_qb: emit_proj_half(q - 2, 0, half=0))
            _add(_qb, 9, lambda q=_qb: emit_proj_half(q - 2, 0, half=1))
            _add(_qb, 11, lambda q=_qb: emit_proj_half(q - 2, 1, half=0))
            _add(_qb, 12, lambda q=_qb: emit_proj_half(q - 2, 1, half=1))

        def emit_scores_pair(ps_sc, qb, kt, pair):
            qs = qb * QW
            for hh in range(2):
                h = 2 * pair + hh
                hp = (HD * h) % P                 # 0, 64, 0, 64
                hc = h // 2                       # q ctile; k ctile = 2 + hc
                nc.tensor.matmul(
                    ps_sc[:, hh * 512:(hh + 1) * 512],
                    lhsT=qkT_sb[hp:hp + HD, 2 + hc, kt * P:(kt + 1) * P],
                    rhs=qkT_sb[hp:hp + HD, hc, qs:qs + QW],
                    start=True,
                    stop=True,
                )

        def new_sc():
            # one 2-bank PSUM tile per head pair: exp(pair, kt) and
            # scores(pair, kt+1) of DIFFERENT pairs overlap, breaking the
            # full-tile exp<->scores serialization of a single 4-bank tile
            return (ps_s.tile([P, 1024], F32, name="ps_sc0", tag="sc0"),
                    ps_s.tile([P, 1024], F32, name="ps_sc1", tag="sc1"))

        # first scores pair right after its two qk groups; pair 1's qk
        # groups land while ScalarE already runs expA(kt0)
        sc_next = new_sc()
        emit_scores_pair(sc_next[0], 0, 0, 0)
        emit_qk(0, 3, pre=True)
        emit_qk(0, 1, pre=True)
        emit_scores_pair(sc_next[1], 0, 0, 1)
        for qb in range(QB):
            qs = qb * QW
            lag = 1 if qb == QB - 1 else 3
            ps_av0 = ps_av1 = ps_rs = None
            pending = []
            eacc = {}

            if sc_next is None:
                sc_next = new_sc()
                emit_scores_pair(sc_next[0], qb, 0, 0)
                emit_scores_pair(sc_next[1], qb, 0, 1)
            sc_cur = sc_next
            sc_next = None

            for kt in range(KT):
                e_sb = ep.tile([P, 4 * 512], BF16, name="e_sb")
                nc.scalar.activation(e_sb[:, 0:1024], sc_cur[0][:], exp_fn,
                                     scale=SCALE)
                nc.scalar.activation(e_sb[:, 1024:2048], sc_cur[1][:],
                                     exp_fn, scale=SCALE)
                # group-accumulate exp tiles on DVE (groups kt 0-7 and
                # 8-14; kt15 is summed directly at the boundary)
                g = kt // 8
                if kt % 8 == 0 and kt < 15:
                    eacc[g] = eap.tile([P, 4 * 512], BF16, name="eacc")
                    nc.vector.tensor_copy(eacc[g][:], e_sb[:])
                elif kt < 15:
                    nc.vector.tensor_add(eacc[g][:], eacc[g][:], e_sb[:])
                nxt = kt + 1 < KT
                if nxt:
                    sc_nx = new_sc()
                # lagged av first: its operands are ready, so the PE works
                # through it while ScalarE is still on this tile's exp
                pending.append((kt, e_sb))
                if len(pending) > lag:
                    if ps_av0 is None:
                        ps_av0 = ps_acc.tile([P, 512], F32, tag="acc",
                                             name="ps_av0")
                        ps_av1 = ps_acc.tile([P, 512], F32, tag="acc",
                                             name="ps_av1")
                        ps_rs = ps_acc.tile([P, 512], F32, tag="acc",
                                            name="ps_rs")
                    k0, e0 = pending.pop(0)
                    emit_av_pair(k0, e0, ps_av0, 0)
                    emit_av_pair(k0, e0, ps_av1, 1)
                if kt in (8, 15):
                    gg = 0 if kt == 8 else 1
                    emit_rs_acc(eacc[gg], ps_rs, start=(gg == 0), stop=False)
                if nxt:
                    emit_scores_pair(sc_nx[0], qb, kt + 1, 0)
                    emit_scores_pair(sc_nx[1], qb, kt + 1, 1)
                for fn in filler[qb].get(kt, ()):
                    fn()
                if nxt:
                    sc_cur = sc_nx
            e15 = pending[-1][1]
            for k0, e0 in pending:
                emit_av_pair(k0, e0, ps_av0, 0)
                emit_av_pair(k0, e0, ps_av1, 1)
            emit_rs_acc(e15, ps_rs, start=False, stop=True)

            if qb + 1 < QB:
                sc_next = new_sc()
                emit_scores_pair(sc_next[0], qb + 1, 0, 0)
                emit_scores_pair(sc_next[1], qb + 1, 0, 1)

            # release the accumulator PSUM slots fast: raw copies to SBUF
            o_sb = [rp.tile([P, 512], BF16, name="o0_sb"),
                    rp.tile([P, 512], BF16, name="o1_sb")]
            if qb == QB - 1:   # tail: ScalarE is idle, DVE does recip in parallel
                nc.scalar.copy(o_sb[0][:], ps_av0[:])
                nc.scalar.copy(o_sb[1][:], ps_av1[:])
            else:
                nc.vector.tensor_copy(o_sb[0][:], ps_av0[:])
                nc.vector.tensor_copy(o_sb[1][:], ps_av1[:])
            rf_sb = rp.tile([P, 512], F32, name="rf_sb")
            nc.vector.reciprocal_approx_fast(out=rf_sb[:], in_=ps_rs[:])
            r_sb = rp.tile([P, 512], BF16, name="r_sb")
            nc.vector.tensor_copy(r_sb[:], rf_sb[:])

            if qb < QB - 1:
                _add(qb + 1, 1, make_norm_pair(qb, 0, o_sb[0], r_sb))
                _add(qb + 1, 1, make_ag_half(qb, 0))
                _add(qb + 1, 2, make_norm_pair(qb, 1, o_sb[1], r_sb))
                _add(qb + 1, 2, make_ag_half(qb, 1))
            else:
                make_norm_pair(qb, 0, o_sb[0], r_sb)()
                make_ag_half(qb, 0)()
                make_norm_pair(qb, 1, o_sb[1], r_sb)()
                make_ag_half(qb, 1)()

        emit_proj_half(QB - 2, 0)
        emit_proj_half(QB - 2, 1)
        # both output halves' even-kd accumulations first (they need only
        # the j=0 gather), then the odd halves once the j=1 gather lands
        emit_proj_half(QB - 1, 0, half=0, tail=True)
        emit_proj_half(QB - 1, 1, half=0, tail=True)
        emit_proj_half(QB - 1, 0, half=1)
        emit_proj_half(QB - 1, 1, half=1)


def _build():
    if "nc" in _CACHE:
        return _CACHE["nc"]
    nc = bacc.Bacc(
        "TRN2",
        target_bir_lowering=False,
        debug=False,
        num_devices=NCORES,
    )
    xT = nc.declare_dram_parameter("xT", [QB, P, KD * 512], BF16, isOutput=False)
    wqk = nc.declare_dram_parameter("wqk", [P, KD * 2 * DL], BF16, isOutput=False)
    wv = nc.declare_dram_parameter("wv", [P, KD * DL], BF16, isOutput=False)
    wp = nc.declare_dram_parameter("wp", [P, KD * DL], BF16, isOutput=False)
    bqk = nc.declare_dram_parameter("bqk", [P, 4], F32, isOutput=False)
    beff = nc.declare_dram_parameter("beff", [P, 2], F32, isOutput=False)
    yT = nc.declare_dram_parameter("yT", [DL, S], F32, isOutput=True)

    with tile.TileContext(nc) as tc:
        _emit(nc, tc, xT, wqk, wv, wp, bqk, beff, yT)
    nc.compile()
    _CACHE["nc"] = nc
    return nc


def kernel(x, W_qkv, b_qkv, W_proj, b_proj):
    x = np.asarray(x, np.float32)
    W_qkv = np.asarray(W_qkv, np.float32)
    b_qkv = np.asarray(b_qkv, np.float32)
    W_proj = np.asarray(W_proj, np.float32)
    b_proj = np.asarray(b_proj, np.float32)

    nc = _build()

    b_v = b_qkv[2 * D:3 * D]
    xTt = {}
    for b in range(B):
        xT = np.ascontiguousarray(x[b].T)            # [1024, 2048]
        t = xT.reshape(KD, P, QB, 512).transpose(2, 1, 0, 3)
        xTt[b] = np.ascontiguousarray(t.reshape(QB, P, KD * 512)).astype(NBF16)

    in_maps = []
    for c in range(NCORES):
        b, g = divmod(c, 4)
        cs = DL * g
        wqk_c = np.concatenate(
            [W_qkv[:, cs:cs + DL], W_qkv[:, D + cs:D + cs + DL]], axis=1)
        # pack ct-major ([q0,q1,k0,k1] column blocks of 128) so the device
        # can DMA each block separately, earliest-needed first
        wqk_p = np.concatenate(
            [_restripe(wqk_c[:, ct * P:(ct + 1) * P]) for ct in range(4)],
            axis=1)
        bqk_c = np.concatenate(
            [b_qkv[cs:cs + DL], b_qkv[D + cs:D + cs + DL]]).reshape(4, P).T
        beff_c = (b_v @ W_proj[:, cs:cs + DL] + b_proj[cs:cs + DL]).reshape(2, P).T
        in_maps.append({
            "xT": xTt[b],
            "wqk": np.ascontiguousarray(wqk_p).astype(NBF16),
            "wv": _restripe(W_qkv[:, 2 * D + cs:2 * D + cs + DL]).astype(NBF16),
            "wp": _restripe(W_proj[:, cs:cs + DL]).astype(NBF16),
            "bqk": np.ascontiguousarray(bqk_c, np.float32),
            "beff": np.ascontiguousarray(beff_c, np.float32),
        })

    trace = bool(int(os.environ.get("TRN_KERNEL_TRACE", "0")))
    res = run_bass_kernel_spmd(nc, in_maps, core_ids=list(range(NCORES)),
                               trace=trace)
    if trace and res.exec_time_ns is not None:
        print(f"HW exec time: {res.exec_time_ns} ns", flush=True)
    _CACHE["last_result"] = res

    out = np.empty((B, S, D), np.float32)
    for c in range(NCORES):
        b, g = divmod(c, 4)
        out[b, :, DL * g:DL * (g + 1)] = res.results[c]["yT"].T
    return out
